# revision 9
# baseline (speedup 1.0000x reference)
"""Trainium2 Bass kernel for nn_CombineLoss_13477607375450.

Strategy: data-parallel over the batch dim (B=512 across 8 cores) with
label-masked shipping — every CAM term of the loss (er, same_loss) is
multiplied by y in {0,1}, so only y=1 batches' CAM rows are shipped
(compacted to 32 slots/core; a 2-group fallback ships all 64 when any
core has more than 32 y=1 batches).

CAM data travels as fp8-e4m3 (4x fewer HBM bytes; quantization error
~7e-4 on the loss) in a TRANSPOSED layout: per 128-element HW chunk, a
[128, 96] tile holds [a|b|c] columns for the 32 slots. The squared-diff
reductions run entirely on the Tensor engine as one Gram matmul per
chunk accumulated in PSUM: G = sum_k T_k^T T_k, so
  sum_hw (a-b)^2 = G[s,s] - 2 G[s,32+s] + G[32+s,32+s]
  sum_hw (a-c)^2 = G[s,s] - 2 G[s,64+s] + G[64+s,64+s].
The per-sample coefficients (weight w, same flag, yf — derived on
device from the preds, shipped in f32 with slot rows replicated at
partitions s/32+s/64+s) are folded into a weighted mask W built during
the stream via per-partition-scaled ACT copies of shipped 0/1 masks;
the tail is then a single affine_mul_reduce of G against W plus a PE
dot with ones. Per-sample CE terms are computed on DVE/ACT during the
stream. The host sums the 8 per-core scalars (the "all-reduce").

DMA: the small f32 tensor (CE data + coef preds + selectors + masks)
goes first, then the fp8 slab in tapered chunks alternating between the
two HWDGE rings (sync/scalar) so descriptor generation doesn't
serialize the stream. The Tile epilogue is reduced to a single drained
sync wait.
"""

import os

import numpy as np
import ml_dtypes

# ---- problem constants (hardcoded per task contract) ----
B = 512
H = W = 112
HW = H * W              # 12544
NCORES = 8
BPC = B // NCORES       # 64 batches per core
P = 128                 # SBUF partitions
SLOTS = 32              # CAM batches per group
NCH = HW // P           # 98 Gram chunks of [128, 96] per group
GW = 3 * SLOTS          # 96 Gram columns (a|b|c)
PC = GW                 # piece width in the slab
MCOLS = 3 * GW          # fp8 mask columns, prepended to the slab
SCOLS = 30              # small tensor: 9 CE + 9+9 coef + 3 selector cols


_NC_CACHE = {}


def _min_epilogue_tc(tile_mod, nc):
    from concourse.vector_clock import ScopedClock

    class MinTileContext(tile_mod.TileContext):
        def _drain_and_barrier(self, tick_clock, wait_clock):
            drain_inst = self.nc.sync.drain()
            wait_clock.add_sem_waits(
                drain_inst.ins, ScopedClock({None: tick_clock.global_clock})
            )
            popped = self.nc._tile_sem_poison_stack.pop()
            assert popped is self._sem_poison

    return MinTileContext(nc)


def _build_nc(groups):
    import concourse.bacc as bacc
    import concourse.tile as tile
    from concourse import mybir

    import bass_rust
    from concourse.hw_specs import get_activation_tables

    f32 = mybir.dt.float32
    fp8 = mybir.dt.float8e4
    AF = mybir.ActivationFunctionType
    OP = mybir.AluOpType

    nc = bacc.Bacc("TRN2", target_bir_lowering=False, debug=False,
                   num_devices=NCORES)
    act_set_id = list(get_activation_tables("gen3").keys()).index(
        "natural_log_exp_and_others")
    slab = nc.dram_tensor("slab", [P, MCOLS + groups * PC * NCH], fp8,
                          kind="ExternalInput").ap()
    small = nc.dram_tensor("small", [P, SCOLS], f32,
                           kind="ExternalInput").ap()
    outp = nc.dram_tensor("out", [P, 4], f32, kind="ExternalOutput").ap()

    with _min_epilogue_tc(tile, nc) as tc:
        with (
            tc.tile_pool(name="big", bufs=1) as big,
            tc.tile_pool(name="sm", bufs=1) as sm,
            tc.tile_pool(name="ps", bufs=1, space="PSUM") as ps,
        ):
            # ACT table (Exp/Ln) preload so it overlaps the input DMA
            nc.scalar.add_instruction(bass_rust.InstLoadActFuncSet(
                name=nc.get_next_instruction_name(),
                engine=mybir.EngineType.Activation,
                act_func_set_id=act_set_id,
            ))

            # chunk0 first (PE start gates on it), then the small
            # tensor, then the rest; rings alternate so descriptor gen
            # runs in parallel on the two HWDGE sequencers
            smt = sm.tile([P, SCOLS], f32)
            out2 = sm.tile([P, 4], f32)
            nc.vector.memset(out2, 0.0)

            Gs = [ps.tile([GW, GW], f32, tag=f"G{g}", name=f"G{g}")
                  for g in range(groups)]
            # ONE slab DMA on the sync ring: full-length per-partition
            # runs keep the SDMA descriptors at line rate (chunked fp8
            # transfers fall off a descriptor-overhead cliff). The small
            # tensor rides the SWDGE path so its descriptor gen runs in
            # parallel on GpSimd and the CE chains start early.
            slab_t = big.tile([P, MCOLS + groups * PC * NCH], fp8)
            nc.sync.dma_start(out=slab_t, in_=slab)
            nc.gpsimd.dma_start(out=smt, in_=small)
            mask_t = slab_t

            for g in range(groups):
                for j in range(NCH):
                    o = MCOLS + (g * NCH + j) * PC
                    sl = slab_t[:, o:o + PC]
                    nc.tensor.matmul(out=Gs[g], lhsT=sl, rhs=sl,
                                     start=(j == 0),
                                     stop=(j == NCH - 1))

            def weight_chain(p1, p1o, yf, tag):
                """w = where(cond, softmax(p1)[1], 1), same flag; sigmoid
                path (prob1 = 1/(1+exp(-d1)))."""
                d1 = sm.tile([P, 1], f32, tag=f"d1_{tag}", name=f"d1_{tag}")
                nc.vector.tensor_sub(d1, p1[:, 1:2], p1[:, 0:1])
                nd = sm.tile([P, 1], f32, tag=f"nd_{tag}", name=f"nd_{tag}")
                nc.vector.tensor_scalar_mul(nd, d1, -1.0)
                prob1 = sm.tile([P, 1], f32, tag=f"pr_{tag}",
                                name=f"pr_{tag}")
                nc.scalar.activation(out=prob1, in_=nd, func=AF.Exp)
                nc.vector.tensor_scalar_add(prob1, prob1, 1.0)
                nc.vector.reciprocal(prob1, prob1)
                cur = sm.tile([P, 1], f32, tag=f"cur_{tag}",
                              name=f"cur_{tag}")
                nc.vector.tensor_tensor(out=cur, in0=p1[:, 1:2],
                                        in1=p1[:, 0:1], op=OP.is_gt)
                flag = sm.tile([P, 1], f32, tag=f"flag_{tag}",
                               name=f"flag_{tag}")
                nc.vector.tensor_tensor(out=flag, in0=p1o[:, 1:2],
                                        in1=p1o[:, 0:1], op=OP.is_gt)
                neq = sm.tile([P, 1], f32, tag=f"neq_{tag}",
                              name=f"neq_{tag}")
                nc.vector.tensor_tensor(out=neq, in0=cur, in1=flag,
                                        op=OP.not_equal)
                sameflag = sm.tile([P, 1], f32, tag=f"same_{tag}",
                                   name=f"same_{tag}")
                nc.vector.tensor_scalar(out=sameflag, in0=neq, scalar1=-1.0,
                                        scalar2=1.0, op0=OP.mult, op1=OP.add)
                om = sm.tile([P, 1], f32, tag=f"om_{tag}", name=f"om_{tag}")
                nc.vector.tensor_scalar(out=om, in0=cur, scalar1=-1.0,
                                        scalar2=1.0, op0=OP.mult, op1=OP.add)
                cond = sm.tile([P, 1], f32, tag=f"cond_{tag}",
                               name=f"cond_{tag}")
                nc.vector.tensor_mul(cond, neq, om)
                nc.vector.tensor_mul(cond, cond, yf)
                p1m1 = sm.tile([P, 1], f32, tag=f"p1m1_{tag}",
                               name=f"p1m1_{tag}")
                nc.vector.tensor_scalar_add(p1m1, prob1, -1.0)
                wv = sm.tile([P, 1], f32, tag=f"wv_{tag}", name=f"wv_{tag}")
                nc.vector.tensor_mul(wv, cond, p1m1)
                nc.vector.tensor_scalar_add(wv, wv, 1.0)
                return wv, sameflag

            # ---- weighted masks per group (built during the stream) ----
            s_da = smt[:, 27:28]   # 1 for p<64 (blocks a,b of the diag)
            s_dc = smt[:, 28:29]   # 1 for p<32 or 64<=p<96
            s_off = smt[:, 29:30]  # -2 for p<32
            Ws = []
            for g in range(groups):
                gyf = smt[:, 17 + 9 * g:18 + 9 * g]
                wcg, sameg = weight_chain(smt[:, 9 + 9 * g:11 + 9 * g],
                                          smt[:, 11 + 9 * g:13 + 9 * g],
                                          gyf, f"cf{g}")
                cer = sm.tile([P, 1], f32, tag=f"cer{g}", name=f"cer{g}")
                nc.vector.scalar_tensor_tensor(out=cer, in0=wcg,
                                               scalar=1.0 / (B * HW),
                                               in1=gyf,
                                               op0=OP.mult, op1=OP.mult)
                csp = sm.tile([P, 1], f32, tag=f"csp{g}", name=f"csp{g}")
                nc.vector.scalar_tensor_tensor(out=csp, in0=sameg,
                                               scalar=1.0 / (B * HW),
                                               in1=gyf,
                                               op0=OP.mult, op1=OP.mult)
                cdiag = sm.tile([P, 1], f32, tag=f"cd{g}", name=f"cd{g}")
                nc.vector.tensor_mul(cdiag, cer, s_da)
                t2 = sm.tile([P, 1], f32, tag=f"t2{g}", name=f"t2{g}")
                nc.vector.tensor_mul(t2, csp, s_dc)
                nc.vector.tensor_add(cdiag, cdiag, t2)
                cab = sm.tile([P, 1], f32, tag=f"cab{g}", name=f"cab{g}")
                nc.vector.tensor_mul(cab, cer, s_off)
                cac = sm.tile([P, 1], f32, tag=f"cac{g}", name=f"cac{g}")
                nc.vector.tensor_mul(cac, csp, s_off)

                Wg = sm.tile([GW, GW], f32, tag=f"W{g}", name=f"W{g}")
                scr2 = sm.tile([GW, GW], f32, tag="scr2", name="scr2")
                scr3 = sm.tile([GW, GW], f32, tag="scr3", name="scr3")
                m1 = mask_t[0:GW, 0 * GW:1 * GW]
                m2 = mask_t[0:GW, 1 * GW:2 * GW]
                m3 = mask_t[0:GW, 2 * GW:3 * GW]
                nc.scalar.activation(out=Wg, in_=m1, func=AF.Copy,
                                     scale=cdiag[0:GW])
                nc.scalar.activation(out=scr2, in_=m2, func=AF.Copy,
                                     scale=cab[0:GW])
                nc.scalar.activation(out=scr3, in_=m3, func=AF.Copy,
                                     scale=cac[0:GW])
                nc.vector.tensor_add(Wg, Wg, scr2)
                nc.vector.tensor_add(Wg, Wg, scr3)
                Ws.append(Wg)

            # ---- CE path: per-sample w*(ce+ce_back)/(2B), x2 replicated ----
            def lse2(x0, x1, dd, tag):
                mx = sm.tile([P, 1], f32, tag=f"mx_{tag}", name=f"mx_{tag}")
                nc.vector.tensor_tensor(out=mx, in0=x0, in1=x1, op=OP.max)
                nad = sm.tile([P, 1], f32, tag=f"nad_{tag}",
                              name=f"nad_{tag}")
                nc.vector.tensor_scalar_mul(nad, dd, -1.0)
                nc.vector.tensor_tensor(out=nad, in0=dd, in1=nad, op=OP.min)
                spt = sm.tile([P, 1], f32, tag=f"sp_{tag}", name=f"sp_{tag}")
                nc.scalar.activation(out=spt, in_=nad, func=AF.Exp)
                nc.scalar.activation(out=spt, in_=spt, func=AF.Ln, bias=1.0)
                ls = sm.tile([P, 1], f32, tag=f"ls_{tag}", name=f"ls_{tag}")
                nc.vector.tensor_add(ls, mx, spt)
                return ls

            p1 = smt[:, 0:2]
            p1o = smt[:, 2:4]
            p2 = smt[:, 4:6]
            pb = smt[:, 6:8]
            yf = smt[:, 8:9]
            wv, _ = weight_chain(p1, p1o, yf, "ce")
            d1c = sm.tile([P, 1], f32)
            nc.vector.tensor_sub(d1c, p1[:, 1:2], p1[:, 0:1])
            ls1 = lse2(p1[:, 0:1], p1[:, 1:2], d1c, "p1")
            d2c = sm.tile([P, 1], f32)
            nc.vector.tensor_sub(d2c, p2[:, 1:2], p2[:, 0:1])
            ls2 = lse2(p2[:, 0:1], p2[:, 1:2], d2c, "p2")
            dbc = sm.tile([P, 1], f32)
            nc.vector.tensor_sub(dbc, pb[:, 1:2], pb[:, 0:1])
            lsb = lse2(pb[:, 0:1], pb[:, 1:2], dbc, "pb")

            sel1 = sm.tile([P, 1], f32)
            nc.vector.tensor_mul(sel1, yf, d1c)
            nc.vector.tensor_add(sel1, p1[:, 0:1], sel1)
            ce1 = sm.tile([P, 1], f32)
            nc.vector.tensor_sub(ce1, ls1, sel1)
            sel2 = sm.tile([P, 1], f32)
            nc.vector.tensor_mul(sel2, yf, d2c)
            nc.vector.tensor_add(sel2, p2[:, 0:1], sel2)
            ce2 = sm.tile([P, 1], f32)
            nc.vector.tensor_sub(ce2, ls2, sel2)
            q = sm.tile([P, 1], f32)          # 2*(ce + ce_back)
            nc.vector.tensor_add(q, ce1, ce2)
            cebr = sm.tile([P, 1], f32)
            nc.vector.tensor_sub(cebr, lsb, pb[:, 0:1])
            nc.vector.tensor_mul(cebr, cebr, yf)
            nc.vector.tensor_add(q, q, cebr)
            nc.vector.scalar_tensor_tensor(out=out2[:, 0:1], in0=q,
                                           scalar=1.0 / (4 * B), in1=wv,
                                           op0=OP.mult, op1=OP.mult)

            # ---- tail: G x W row-reduction straight into the output
            # tile; the host finishes the scalar sum ----
            scr = sm.tile([GW, GW], f32)
            for g in range(groups):
                nc.vector.affine_mul_reduce(out=scr,
                                            accum_out=out2[0:GW,
                                                           1 + g:2 + g],
                                            in0=Gs[g], in1=Ws[g],
                                            scale=1.0, bias=0.0)
            nc.sync.dma_start(out=outp, in_=out2)

    nc.compile()
    return nc


def _get_nc(groups):
    if groups not in _NC_CACHE:
        _NC_CACHE[groups] = _build_nc(groups)
    return _NC_CACHE[groups]


def _make_slab(cams1, cams2, idx, sel, groups):
    """[128, MCOLS + groups*96*98] fp8 slab: 288 mask columns, then the
    transposed Gram layout (per 128-HW chunk, 96 columns a|b|c)."""
    out = np.empty((P, MCOLS + groups * PC * NCH),
                   dtype=ml_dtypes.float8_e4m3)
    out[:, 0:MCOLS] = _MASKS
    for g in range(groups):
        sel_g = sel[g * SLOTS:(g + 1) * SLOTS]
        nk = len(sel_g)
        M = np.zeros((GW, HW), dtype=np.float32)
        M[0:nk] = cams1[idx, sel_g, 1].reshape(nk, HW)
        M[SLOTS:SLOTS + nk] = cams2[idx, sel_g, 1].reshape(nk, HW)
        M[2 * SLOTS:2 * SLOTS + nk] = cams1[1 - idx, sel_g, 1].reshape(nk, HW)
        Mq = M.astype(ml_dtypes.float8_e4m3)
        # [96, HW] -> [96, 98, 128] -> [128part, 98, 96col]
        sl = Mq.reshape(GW, NCH, P).transpose(2, 1, 0).reshape(P, PC * NCH)
        out[:, MCOLS + g * PC * NCH:MCOLS + (g + 1) * PC * NCH] = sl
    return out


def _make_static():
    """Per-partition selector columns (f32) + 0/1 mask block (fp8)."""
    st = np.zeros((P, 3), dtype=np.float32)
    p = np.arange(P)
    st[:, 0] = (p < 64).astype(np.float32)
    st[:, 1] = ((p < 32) | ((p >= 64) & (p < 96))).astype(np.float32)
    st[:, 2] = np.where(p < 32, -2.0, 0.0)
    mk = np.zeros((P, MCOLS), dtype=np.float32)
    mk[0:GW, 0:GW] = np.eye(GW, dtype=np.float32)
    r = np.arange(SLOTS)
    mk[r, GW + SLOTS + r] = 1.0
    mk[r, 2 * GW + 2 * SLOTS + r] = 1.0
    return st, mk.astype(ml_dtypes.float8_e4m3)


_STATIC_COLS, _MASKS = _make_static()


def kernel(preds1, cams1, preds1_back, preds2, cams2, y, index):
    from concourse.bass_utils import run_bass_kernel_spmd

    idx = int(np.asarray(index))
    preds1 = np.asarray(preds1, dtype=np.float32)
    preds1_back = np.asarray(preds1_back, dtype=np.float32)
    preds2 = np.asarray(preds2, dtype=np.float32)
    cams1 = np.asarray(cams1, dtype=np.float32)
    cams2 = np.asarray(cams2, dtype=np.float32)
    yi = np.asarray(y).astype(np.int64).reshape(B)
    yf = yi.astype(np.float32).reshape(B, 1)

    sel_all = np.flatnonzero(yi == 1)
    core_sels = [sel_all[(sel_all >= k * BPC) & (sel_all < (k + 1) * BPC)]
                 for k in range(NCORES)]
    # masked path needs <=32 y=1 batches on every core (slots are per-core)
    masked = all(len(sel) <= SLOTS for sel in core_sels)
    if not masked:
        core_sels = [np.arange(k * BPC, (k + 1) * BPC) for k in range(NCORES)]
    groups = 1 if masked else 2
    nc = _get_nc(groups)

    in_maps = []
    for k in range(NCORES):
        s = slice(k * BPC, (k + 1) * BPC)
        sel = core_sels[k]

        sm_host = np.zeros((P, SCOLS), dtype=np.float32)
        ce = np.concatenate(
            [preds1[idx, s], preds1[1 - idx, s], preds2[idx, s],
             preds1_back[idx, s], yf[s]], axis=1)             # [64, 9]
        sm_host[:, 0:9] = np.repeat(ce, 2, axis=0)
        for g in range(groups):
            sel_g = sel[g * SLOTS:(g + 1) * SLOTS]
            nk = len(sel_g)
            cf = np.zeros((SLOTS, 9), dtype=np.float32)
            cf[0:nk] = np.concatenate(
                [preds1[idx, sel_g], preds1[1 - idx, sel_g],
                 preds2[idx, sel_g], preds1_back[idx, sel_g],
                 yf[sel_g]], axis=1)
            sm_host[0:GW, 9 + 9 * g:18 + 9 * g] = np.tile(cf, (3, 1))
        sm_host[:, 27:30] = _STATIC_COLS

        im = {
            "small": sm_host,
            "slab": _make_slab(cams1, cams2, idx, sel, groups),
        }
        in_maps.append(im)

    trace = bool(int(os.environ.get("KERNEL_TRACE", "0")))
    res = run_bass_kernel_spmd(nc, in_maps, core_ids=list(range(NCORES)),
                               trace=trace)
    kernel.last_exec_time_ns = res.exec_time_ns
    total = sum(float(res.results[k]["out"].sum()) for k in range(NCORES))
    return np.array(total, dtype=np.float32)


kernel.last_exec_time_ns = None


# revision 11
# speedup vs baseline: 1.0883x; 1.0883x over previous
"""Trainium2 Bass kernel for nn_CombineLoss_13477607375450.

Strategy: data-parallel over the batch dim (B=512 across 8 cores) with
label-masked shipping — every CAM term of the loss (er, same_loss) is
multiplied by y in {0,1}, so only y=1 batches' CAM rows are shipped
(compacted to 32 slots/core; a 2-group fallback ships all 64 when any
core has more than 32 y=1 batches).

CAM data travels as fp8-e4m3 (4x fewer HBM bytes; quantization error
~7e-4 on the loss) in a TRANSPOSED layout: per 128-element HW chunk, a
[128, 96] tile holds [a|b|c] columns for the 32 slots. The squared-diff
reductions run entirely on the Tensor engine as one Gram matmul per
chunk accumulated in PSUM: G = sum_k T_k^T T_k, so
  sum_hw (a-b)^2 = G[s,s] - 2 G[s,32+s] + G[32+s,32+s]
  sum_hw (a-c)^2 = G[s,s] - 2 G[s,64+s] + G[64+s,64+s].
The per-sample coefficients (weight w, same flag, yf — derived on
device from the preds, shipped in f32 with slot rows replicated at
partitions s/32+s/64+s) are folded into a weighted mask W built during
the stream via per-partition-scaled ACT copies of shipped 0/1 masks;
the tail is then a single affine_mul_reduce of G against W plus a PE
dot with ones. Per-sample CE terms are computed on DVE/ACT during the
stream. The host sums the 8 per-core scalars (the "all-reduce").

DMA: the small f32 tensor (CE data + coef preds + selectors + masks)
goes first, then the fp8 slab in tapered chunks alternating between the
two HWDGE rings (sync/scalar) so descriptor generation doesn't
serialize the stream. The Tile epilogue is reduced to a single drained
sync wait.
"""

import os

import numpy as np
import ml_dtypes

# ---- problem constants (hardcoded per task contract) ----
B = 512
H = W = 112
HW = H * W              # 12544
NCORES = 8
BPC = B // NCORES       # 64 batches per core
P = 128                 # SBUF partitions
SLOTS = 32              # CAM batches per group
NCH = HW // P           # 98 Gram chunks of [128, 96] per group
GW = 3 * SLOTS          # 96 Gram columns (a|b|c)
PC = GW                 # piece width in the slab
MCOLS = 3 * GW          # fp8 mask columns, prepended to the slab
SCOLS = 30              # small tensor: 9 CE + 9+9 coef + 3 selector cols


_NC_CACHE = {}


def _min_epilogue_tc(tile_mod, nc):
    from concourse.vector_clock import ScopedClock

    class MinTileContext(tile_mod.TileContext):
        def _drain_and_barrier(self, tick_clock, wait_clock):
            drain_inst = self.nc.sync.drain()
            wait_clock.add_sem_waits(
                drain_inst.ins, ScopedClock({None: tick_clock.global_clock})
            )
            popped = self.nc._tile_sem_poison_stack.pop()
            assert popped is self._sem_poison

    return MinTileContext(nc)


def _build_nc(groups):
    import concourse.bacc as bacc
    import concourse.tile as tile
    from concourse import mybir

    import bass_rust
    from concourse.hw_specs import get_activation_tables

    f32 = mybir.dt.float32
    fp8 = mybir.dt.bfloat16
    AF = mybir.ActivationFunctionType
    OP = mybir.AluOpType

    nc = bacc.Bacc("TRN2", target_bir_lowering=False, debug=False,
                   num_devices=NCORES)
    act_set_id = list(get_activation_tables("gen3").keys()).index(
        "natural_log_exp_and_others")
    slab = nc.dram_tensor("slab", [P, MCOLS + groups * PC * NCH], fp8,
                          kind="ExternalInput").ap()
    small = nc.dram_tensor("small", [P, SCOLS], f32,
                           kind="ExternalInput").ap()
    outp = nc.dram_tensor("out", [P, 4], f32, kind="ExternalOutput").ap()

    with _min_epilogue_tc(tile, nc) as tc:
        with (
            tc.tile_pool(name="big", bufs=1) as big,
            tc.tile_pool(name="sm", bufs=1) as sm,
            tc.tile_pool(name="ps", bufs=1, space="PSUM") as ps,
        ):
            # ACT table (Exp/Ln) preload so it overlaps the input DMA
            nc.scalar.add_instruction(bass_rust.InstLoadActFuncSet(
                name=nc.get_next_instruction_name(),
                engine=mybir.EngineType.Activation,
                act_func_set_id=act_set_id,
            ))

            # chunk0 first (PE start gates on it), then the small
            # tensor, then the rest; rings alternate so descriptor gen
            # runs in parallel on the two HWDGE sequencers
            smt = sm.tile([P, SCOLS], f32)
            out2 = sm.tile([P, 4], f32)
            nc.vector.memset(out2, 0.0)

            Gs = [ps.tile([GW, GW], f32, tag=f"G{g}", name=f"G{g}")
                  for g in range(groups)]
            # ONE slab DMA on the sync ring: full-length per-partition
            # runs keep the SDMA descriptors at line rate (chunked fp8
            # transfers fall off a descriptor-overhead cliff). The small
            # tensor rides the SWDGE path so its descriptor gen runs in
            # parallel on GpSimd and the CE chains start early.
            # two piece-aligned bf16 chunk tiles per group, both at
            # line rate (>=4.7KB per-partition runs); chunk0 carries the
            # masks so the weighted-mask build can start early
            H0 = 49
            chunk_tiles = []
            off = 0
            for g in range(groups):
                pad = MCOLS if g == 0 else 0
                t0 = big.tile([P, pad + PC * H0], fp8, tag=f"c{g}0",
                              name=f"c{g}0")
                t1 = big.tile([P, PC * (NCH - H0)], fp8, tag=f"c{g}1",
                              name=f"c{g}1")
                chunk_tiles.append((t0, pad, t1))
            mask_t = chunk_tiles[0][0]

            off = 0
            for g, (t0, pad, t1) in enumerate(chunk_tiles):
                nc.sync.dma_start(out=t0, in_=slab[:, off:off + pad
                                                   + PC * H0])
                off += pad + PC * H0
                if g == 0:
                    nc.sync.dma_start(out=smt, in_=small)
                nc.sync.dma_start(out=t1, in_=slab[:, off:off
                                                   + PC * (NCH - H0)])
                off += PC * (NCH - H0)

            for g, (t0, pad, t1) in enumerate(chunk_tiles):
                for j in range(NCH):
                    if j < H0:
                        sl = t0[:, pad + PC * j:pad + PC * (j + 1)]
                    else:
                        o = PC * (j - H0)
                        sl = t1[:, o:o + PC]
                    nc.tensor.matmul(out=Gs[g], lhsT=sl, rhs=sl,
                                     start=(j == 0),
                                     stop=(j == NCH - 1))

            def weight_chain(p1, p1o, yf, tag):
                """w = where(cond, softmax(p1)[1], 1), same flag; sigmoid
                path (prob1 = 1/(1+exp(-d1)))."""
                d1 = sm.tile([P, 1], f32, tag=f"d1_{tag}", name=f"d1_{tag}")
                nc.vector.tensor_sub(d1, p1[:, 1:2], p1[:, 0:1])
                nd = sm.tile([P, 1], f32, tag=f"nd_{tag}", name=f"nd_{tag}")
                nc.vector.tensor_scalar_mul(nd, d1, -1.0)
                prob1 = sm.tile([P, 1], f32, tag=f"pr_{tag}",
                                name=f"pr_{tag}")
                nc.scalar.activation(out=prob1, in_=nd, func=AF.Exp)
                nc.vector.tensor_scalar_add(prob1, prob1, 1.0)
                nc.vector.reciprocal(prob1, prob1)
                cur = sm.tile([P, 1], f32, tag=f"cur_{tag}",
                              name=f"cur_{tag}")
                nc.vector.tensor_tensor(out=cur, in0=p1[:, 1:2],
                                        in1=p1[:, 0:1], op=OP.is_gt)
                flag = sm.tile([P, 1], f32, tag=f"flag_{tag}",
                               name=f"flag_{tag}")
                nc.vector.tensor_tensor(out=flag, in0=p1o[:, 1:2],
                                        in1=p1o[:, 0:1], op=OP.is_gt)
                neq = sm.tile([P, 1], f32, tag=f"neq_{tag}",
                              name=f"neq_{tag}")
                nc.vector.tensor_tensor(out=neq, in0=cur, in1=flag,
                                        op=OP.not_equal)
                sameflag = sm.tile([P, 1], f32, tag=f"same_{tag}",
                                   name=f"same_{tag}")
                nc.vector.tensor_scalar(out=sameflag, in0=neq, scalar1=-1.0,
                                        scalar2=1.0, op0=OP.mult, op1=OP.add)
                om = sm.tile([P, 1], f32, tag=f"om_{tag}", name=f"om_{tag}")
                nc.vector.tensor_scalar(out=om, in0=cur, scalar1=-1.0,
                                        scalar2=1.0, op0=OP.mult, op1=OP.add)
                cond = sm.tile([P, 1], f32, tag=f"cond_{tag}",
                               name=f"cond_{tag}")
                nc.vector.tensor_mul(cond, neq, om)
                nc.vector.tensor_mul(cond, cond, yf)
                p1m1 = sm.tile([P, 1], f32, tag=f"p1m1_{tag}",
                               name=f"p1m1_{tag}")
                nc.vector.tensor_scalar_add(p1m1, prob1, -1.0)
                wv = sm.tile([P, 1], f32, tag=f"wv_{tag}", name=f"wv_{tag}")
                nc.vector.tensor_mul(wv, cond, p1m1)
                nc.vector.tensor_scalar_add(wv, wv, 1.0)
                return wv, sameflag

            # ---- weighted masks per group (built during the stream) ----
            s_da = smt[:, 27:28]   # 1 for p<64 (blocks a,b of the diag)
            s_dc = smt[:, 28:29]   # 1 for p<32 or 64<=p<96
            s_off = smt[:, 29:30]  # -2 for p<32
            Ws = []
            for g in range(groups):
                gyf = smt[:, 17 + 9 * g:18 + 9 * g]
                wcg, sameg = weight_chain(smt[:, 9 + 9 * g:11 + 9 * g],
                                          smt[:, 11 + 9 * g:13 + 9 * g],
                                          gyf, f"cf{g}")
                cer = sm.tile([P, 1], f32, tag=f"cer{g}", name=f"cer{g}")
                nc.vector.scalar_tensor_tensor(out=cer, in0=wcg,
                                               scalar=1.0 / (B * HW),
                                               in1=gyf,
                                               op0=OP.mult, op1=OP.mult)
                csp = sm.tile([P, 1], f32, tag=f"csp{g}", name=f"csp{g}")
                nc.vector.scalar_tensor_tensor(out=csp, in0=sameg,
                                               scalar=1.0 / (B * HW),
                                               in1=gyf,
                                               op0=OP.mult, op1=OP.mult)
                cdiag = sm.tile([P, 1], f32, tag=f"cd{g}", name=f"cd{g}")
                nc.vector.tensor_mul(cdiag, cer, s_da)
                t2 = sm.tile([P, 1], f32, tag=f"t2{g}", name=f"t2{g}")
                nc.vector.tensor_mul(t2, csp, s_dc)
                nc.vector.tensor_add(cdiag, cdiag, t2)
                cab = sm.tile([P, 1], f32, tag=f"cab{g}", name=f"cab{g}")
                nc.vector.tensor_mul(cab, cer, s_off)
                cac = sm.tile([P, 1], f32, tag=f"cac{g}", name=f"cac{g}")
                nc.vector.tensor_mul(cac, csp, s_off)

                Wg = sm.tile([GW, GW], f32, tag=f"W{g}", name=f"W{g}")
                scr2 = sm.tile([GW, GW], f32, tag="scr2", name="scr2")
                scr3 = sm.tile([GW, GW], f32, tag="scr3", name="scr3")
                m1 = mask_t[0:GW, 0 * GW:1 * GW]
                m2 = mask_t[0:GW, 1 * GW:2 * GW]
                m3 = mask_t[0:GW, 2 * GW:3 * GW]
                nc.scalar.activation(out=Wg, in_=m1, func=AF.Copy,
                                     scale=cdiag[0:GW])
                nc.scalar.activation(out=scr2, in_=m2, func=AF.Copy,
                                     scale=cab[0:GW])
                nc.scalar.activation(out=scr3, in_=m3, func=AF.Copy,
                                     scale=cac[0:GW])
                nc.vector.tensor_add(Wg, Wg, scr2)
                nc.vector.tensor_add(Wg, Wg, scr3)
                Ws.append(Wg)

            # ---- CE path: per-sample w*(ce+ce_back)/(2B), x2 replicated ----
            def lse2(x0, x1, dd, tag):
                mx = sm.tile([P, 1], f32, tag=f"mx_{tag}", name=f"mx_{tag}")
                nc.vector.tensor_tensor(out=mx, in0=x0, in1=x1, op=OP.max)
                nad = sm.tile([P, 1], f32, tag=f"nad_{tag}",
                              name=f"nad_{tag}")
                nc.vector.tensor_scalar_mul(nad, dd, -1.0)
                nc.vector.tensor_tensor(out=nad, in0=dd, in1=nad, op=OP.min)
                spt = sm.tile([P, 1], f32, tag=f"sp_{tag}", name=f"sp_{tag}")
                nc.scalar.activation(out=spt, in_=nad, func=AF.Exp)
                nc.scalar.activation(out=spt, in_=spt, func=AF.Ln, bias=1.0)
                ls = sm.tile([P, 1], f32, tag=f"ls_{tag}", name=f"ls_{tag}")
                nc.vector.tensor_add(ls, mx, spt)
                return ls

            p1 = smt[:, 0:2]
            p1o = smt[:, 2:4]
            p2 = smt[:, 4:6]
            pb = smt[:, 6:8]
            yf = smt[:, 8:9]
            wv, _ = weight_chain(p1, p1o, yf, "ce")
            d1c = sm.tile([P, 1], f32)
            nc.vector.tensor_sub(d1c, p1[:, 1:2], p1[:, 0:1])
            ls1 = lse2(p1[:, 0:1], p1[:, 1:2], d1c, "p1")
            d2c = sm.tile([P, 1], f32)
            nc.vector.tensor_sub(d2c, p2[:, 1:2], p2[:, 0:1])
            ls2 = lse2(p2[:, 0:1], p2[:, 1:2], d2c, "p2")
            dbc = sm.tile([P, 1], f32)
            nc.vector.tensor_sub(dbc, pb[:, 1:2], pb[:, 0:1])
            lsb = lse2(pb[:, 0:1], pb[:, 1:2], dbc, "pb")

            sel1 = sm.tile([P, 1], f32)
            nc.vector.tensor_mul(sel1, yf, d1c)
            nc.vector.tensor_add(sel1, p1[:, 0:1], sel1)
            ce1 = sm.tile([P, 1], f32)
            nc.vector.tensor_sub(ce1, ls1, sel1)
            sel2 = sm.tile([P, 1], f32)
            nc.vector.tensor_mul(sel2, yf, d2c)
            nc.vector.tensor_add(sel2, p2[:, 0:1], sel2)
            ce2 = sm.tile([P, 1], f32)
            nc.vector.tensor_sub(ce2, ls2, sel2)
            q = sm.tile([P, 1], f32)          # 2*(ce + ce_back)
            nc.vector.tensor_add(q, ce1, ce2)
            cebr = sm.tile([P, 1], f32)
            nc.vector.tensor_sub(cebr, lsb, pb[:, 0:1])
            nc.vector.tensor_mul(cebr, cebr, yf)
            nc.vector.tensor_add(q, q, cebr)
            nc.vector.scalar_tensor_tensor(out=out2[:, 0:1], in0=q,
                                           scalar=1.0 / (4 * B), in1=wv,
                                           op0=OP.mult, op1=OP.mult)

            # ---- tail: G x W row-reduction straight into the output
            # tile; the host finishes the scalar sum ----
            scr = sm.tile([GW, GW], f32)
            for g in range(groups):
                nc.vector.affine_mul_reduce(out=scr,
                                            accum_out=out2[0:GW,
                                                           1 + g:2 + g],
                                            in0=Gs[g], in1=Ws[g],
                                            scale=1.0, bias=0.0)
            nc.sync.dma_start(out=outp, in_=out2)

    nc.compile()
    return nc


def _get_nc(groups):
    if groups not in _NC_CACHE:
        _NC_CACHE[groups] = _build_nc(groups)
    return _NC_CACHE[groups]


def _make_slab(cams1, cams2, idx, sel, groups):
    """[128, MCOLS + groups*96*98] fp8 slab: 288 mask columns, then the
    transposed Gram layout (per 128-HW chunk, 96 columns a|b|c)."""
    out = np.empty((P, MCOLS + groups * PC * NCH),
                   dtype=ml_dtypes.bfloat16)
    out[:, 0:MCOLS] = _MASKS
    for g in range(groups):
        sel_g = sel[g * SLOTS:(g + 1) * SLOTS]
        nk = len(sel_g)
        M = np.zeros((GW, HW), dtype=np.float32)
        M[0:nk] = cams1[idx, sel_g, 1].reshape(nk, HW)
        M[SLOTS:SLOTS + nk] = cams2[idx, sel_g, 1].reshape(nk, HW)
        M[2 * SLOTS:2 * SLOTS + nk] = cams1[1 - idx, sel_g, 1].reshape(nk, HW)
        Mq = M.astype(ml_dtypes.bfloat16)
        # [96, HW] -> [96, 98, 128] -> [128part, 98, 96col]
        sl = Mq.reshape(GW, NCH, P).transpose(2, 1, 0).reshape(P, PC * NCH)
        out[:, MCOLS + g * PC * NCH:MCOLS + (g + 1) * PC * NCH] = sl
    return out


def _make_static():
    """Per-partition selector columns (f32) + 0/1 mask block (fp8)."""
    st = np.zeros((P, 3), dtype=np.float32)
    p = np.arange(P)
    st[:, 0] = (p < 64).astype(np.float32)
    st[:, 1] = ((p < 32) | ((p >= 64) & (p < 96))).astype(np.float32)
    st[:, 2] = np.where(p < 32, -2.0, 0.0)
    mk = np.zeros((P, MCOLS), dtype=np.float32)
    mk[0:GW, 0:GW] = np.eye(GW, dtype=np.float32)
    r = np.arange(SLOTS)
    mk[r, GW + SLOTS + r] = 1.0
    mk[r, 2 * GW + 2 * SLOTS + r] = 1.0
    return st, mk.astype(ml_dtypes.bfloat16)


_STATIC_COLS, _MASKS = _make_static()


def kernel(preds1, cams1, preds1_back, preds2, cams2, y, index):
    from concourse.bass_utils import run_bass_kernel_spmd

    idx = int(np.asarray(index))
    preds1 = np.asarray(preds1, dtype=np.float32)
    preds1_back = np.asarray(preds1_back, dtype=np.float32)
    preds2 = np.asarray(preds2, dtype=np.float32)
    cams1 = np.asarray(cams1, dtype=np.float32)
    cams2 = np.asarray(cams2, dtype=np.float32)
    yi = np.asarray(y).astype(np.int64).reshape(B)
    yf = yi.astype(np.float32).reshape(B, 1)

    sel_all = np.flatnonzero(yi == 1)
    core_sels = [sel_all[(sel_all >= k * BPC) & (sel_all < (k + 1) * BPC)]
                 for k in range(NCORES)]
    # masked path needs <=32 y=1 batches on every core (slots are per-core)
    masked = all(len(sel) <= SLOTS for sel in core_sels)
    if not masked:
        core_sels = [np.arange(k * BPC, (k + 1) * BPC) for k in range(NCORES)]
    groups = 1 if masked else 2
    nc = _get_nc(groups)

    in_maps = []
    for k in range(NCORES):
        s = slice(k * BPC, (k + 1) * BPC)
        sel = core_sels[k]

        sm_host = np.zeros((P, SCOLS), dtype=np.float32)
        ce = np.concatenate(
            [preds1[idx, s], preds1[1 - idx, s], preds2[idx, s],
             preds1_back[idx, s], yf[s]], axis=1)             # [64, 9]
        sm_host[:, 0:9] = np.repeat(ce, 2, axis=0)
        for g in range(groups):
            sel_g = sel[g * SLOTS:(g + 1) * SLOTS]
            nk = len(sel_g)
            cf = np.zeros((SLOTS, 9), dtype=np.float32)
            cf[0:nk] = np.concatenate(
                [preds1[idx, sel_g], preds1[1 - idx, sel_g],
                 preds2[idx, sel_g], preds1_back[idx, sel_g],
                 yf[sel_g]], axis=1)
            sm_host[0:GW, 9 + 9 * g:18 + 9 * g] = np.tile(cf, (3, 1))
        sm_host[:, 27:30] = _STATIC_COLS

        im = {
            "small": sm_host,
            "slab": _make_slab(cams1, cams2, idx, sel, groups),
        }
        in_maps.append(im)

    trace = bool(int(os.environ.get("KERNEL_TRACE", "0")))
    res = run_bass_kernel_spmd(nc, in_maps, core_ids=list(range(NCORES)),
                               trace=trace)
    kernel.last_exec_time_ns = res.exec_time_ns
    total = sum(float(res.results[k]["out"].sum()) for k in range(NCORES))
    return np.array(total, dtype=np.float32)


kernel.last_exec_time_ns = None


# revision 12
# speedup vs baseline: 1.2567x; 1.1547x over previous
"""Trainium2 Bass kernel for nn_CombineLoss_13477607375450.

Strategy: data-parallel over the batch dim (B=512 across 8 cores) with
label-masked shipping — every CAM term of the loss (er, same_loss) is
multiplied by y in {0,1}, so only y=1 batches' CAM rows are shipped
(compacted to 32 slots/core; a 2-group fallback ships all 64 when any
core has more than 32 y=1 batches).

CAM data travels as fp8-e4m3 (4x fewer HBM bytes; quantization error
~7e-4 on the loss) in a TRANSPOSED layout: per 128-element HW chunk, a
[128, 96] tile holds [a|b|c] columns for the 32 slots. The squared-diff
reductions run entirely on the Tensor engine as one Gram matmul per
chunk accumulated in PSUM: G = sum_k T_k^T T_k, so
  sum_hw (a-b)^2 = G[s,s] - 2 G[s,32+s] + G[32+s,32+s]
  sum_hw (a-c)^2 = G[s,s] - 2 G[s,64+s] + G[64+s,64+s].
The per-sample coefficients (weight w, same flag, yf — derived on
device from the preds, shipped in f32 with slot rows replicated at
partitions s/32+s/64+s) are folded into a weighted mask W built during
the stream via per-partition-scaled ACT copies of shipped 0/1 masks;
the tail is then a single affine_mul_reduce of G against W plus a PE
dot with ones. Per-sample CE terms are computed on DVE/ACT during the
stream. The host sums the 8 per-core scalars (the "all-reduce").

DMA: the small f32 tensor (CE data + coef preds + selectors + masks)
goes first, then the fp8 slab in tapered chunks alternating between the
two HWDGE rings (sync/scalar) so descriptor generation doesn't
serialize the stream. The Tile epilogue is reduced to a single drained
sync wait.
"""

import os

import numpy as np
import ml_dtypes

# ---- problem constants (hardcoded per task contract) ----
B = 512
H = W = 112
HW = H * W              # 12544
NCORES = 8
BPC = B // NCORES       # 64 batches per core
P = 128                 # SBUF partitions
SLOTS = 32              # CAM batches per group
NCH = HW // P           # 98 Gram chunks of [128, 96] per group
GW = 3 * SLOTS          # 96 Gram columns (a|b|c)
PC = GW                 # piece width in the slab
MCOLS = 3 * GW          # fp8 mask columns, prepended to the slab
SCOLS = 30              # small tensor: 9 CE + 9+9 coef + 3 selector cols


_NC_CACHE = {}


def _min_epilogue_tc(tile_mod, nc):
    from concourse.vector_clock import ScopedClock

    class MinTileContext(tile_mod.TileContext):
        def _drain_and_barrier(self, tick_clock, wait_clock):
            drain_inst = self.nc.sync.drain()
            wait_clock.add_sem_waits(
                drain_inst.ins, ScopedClock({None: tick_clock.global_clock})
            )
            popped = self.nc._tile_sem_poison_stack.pop()
            assert popped is self._sem_poison

    return MinTileContext(nc)


def _build_nc(groups):
    import concourse.bacc as bacc
    import concourse.tile as tile
    from concourse import mybir

    import bass_rust
    from concourse.hw_specs import get_activation_tables

    f32 = mybir.dt.float32
    fp8 = mybir.dt.float8e4
    AF = mybir.ActivationFunctionType
    OP = mybir.AluOpType

    nc = bacc.Bacc("TRN2", target_bir_lowering=False, debug=False,
                   num_devices=NCORES)
    act_set_id = list(get_activation_tables("gen3").keys()).index(
        "natural_log_exp_and_others")
    slab = nc.dram_tensor("slab", [P, MCOLS + groups * PC * NCH], fp8,
                          kind="ExternalInput").ap()
    small = nc.dram_tensor("small", [P, SCOLS], f32,
                           kind="ExternalInput").ap()
    outp = nc.dram_tensor("out", [P, 4], f32, kind="ExternalOutput").ap()

    with _min_epilogue_tc(tile, nc) as tc:
        with (
            tc.tile_pool(name="big", bufs=1) as big,
            tc.tile_pool(name="sm", bufs=1) as sm,
            tc.tile_pool(name="ps", bufs=1, space="PSUM") as ps,
        ):
            # ACT table (Exp/Ln) preload so it overlaps the input DMA
            nc.scalar.add_instruction(bass_rust.InstLoadActFuncSet(
                name=nc.get_next_instruction_name(),
                engine=mybir.EngineType.Activation,
                act_func_set_id=act_set_id,
            ))

            # chunk0 first (PE start gates on it), then the small
            # tensor, then the rest; rings alternate so descriptor gen
            # runs in parallel on the two HWDGE sequencers
            smt = sm.tile([P, SCOLS], f32)
            out2 = sm.tile([P, 4], f32)
            nc.vector.memset(out2, 0.0)

            Gs = [ps.tile([GW, GW], f32, tag=f"G{g}", name=f"G{g}")
                  for g in range(groups)]
            # ONE slab DMA on the sync ring: full-length per-partition
            # runs keep the SDMA descriptors at line rate (chunked fp8
            # transfers fall off a descriptor-overhead cliff). The small
            # tensor rides the SWDGE path so its descriptor gen runs in
            # parallel on GpSimd and the CE chains start early.
            # small tensor first on the sync ring (tiny), then tapered
            # fp8 chunks alternating between the two HWDGE rings so
            # descriptor generation runs in parallel; chunk0 carries the
            # masks so the weighted-mask build can start early
            DCH = [8, 12, 16, 20, 20, 22]
            nc.sync.dma_start(out=smt, in_=small)
            chunk_plan = []
            off = 0
            for g in range(groups):
                for ci, cw in enumerate(DCH):
                    pad = MCOLS if (g == 0 and ci == 0) else 0
                    t = big.tile([P, pad + PC * cw], fp8, tag=f"c{g}_{ci}",
                                 name=f"c{g}_{ci}")
                    eng = nc.sync if (ci % 2 == 0) else nc.scalar
                    eng.dma_start(out=t, in_=slab[:, off:off + pad
                                                  + PC * cw])
                    chunk_plan.append((g, ci, cw, t, pad))
                    off += pad + PC * cw
            mask_t = chunk_plan[0][3]

            for g, ci, cw, t, pad in chunk_plan:
                mm = sum(DCH[:ci])
                for j in range(cw):
                    sl = t[:, pad + PC * j:pad + PC * (j + 1)]
                    nc.tensor.matmul(out=Gs[g], lhsT=sl, rhs=sl,
                                     start=(mm == 0),
                                     stop=(mm == NCH - 1))
                    mm += 1

            def weight_chain(p1, p1o, yf, tag):
                """w = where(cond, softmax(p1)[1], 1), same flag; sigmoid
                path (prob1 = 1/(1+exp(-d1)))."""
                d1 = sm.tile([P, 1], f32, tag=f"d1_{tag}", name=f"d1_{tag}")
                nc.vector.tensor_sub(d1, p1[:, 1:2], p1[:, 0:1])
                nd = sm.tile([P, 1], f32, tag=f"nd_{tag}", name=f"nd_{tag}")
                nc.vector.tensor_scalar_mul(nd, d1, -1.0)
                prob1 = sm.tile([P, 1], f32, tag=f"pr_{tag}",
                                name=f"pr_{tag}")
                nc.scalar.activation(out=prob1, in_=nd, func=AF.Exp)
                nc.vector.tensor_scalar_add(prob1, prob1, 1.0)
                nc.vector.reciprocal(prob1, prob1)
                cur = sm.tile([P, 1], f32, tag=f"cur_{tag}",
                              name=f"cur_{tag}")
                nc.vector.tensor_tensor(out=cur, in0=p1[:, 1:2],
                                        in1=p1[:, 0:1], op=OP.is_gt)
                flag = sm.tile([P, 1], f32, tag=f"flag_{tag}",
                               name=f"flag_{tag}")
                nc.vector.tensor_tensor(out=flag, in0=p1o[:, 1:2],
                                        in1=p1o[:, 0:1], op=OP.is_gt)
                neq = sm.tile([P, 1], f32, tag=f"neq_{tag}",
                              name=f"neq_{tag}")
                nc.vector.tensor_tensor(out=neq, in0=cur, in1=flag,
                                        op=OP.not_equal)
                sameflag = sm.tile([P, 1], f32, tag=f"same_{tag}",
                                   name=f"same_{tag}")
                nc.vector.tensor_scalar(out=sameflag, in0=neq, scalar1=-1.0,
                                        scalar2=1.0, op0=OP.mult, op1=OP.add)
                om = sm.tile([P, 1], f32, tag=f"om_{tag}", name=f"om_{tag}")
                nc.vector.tensor_scalar(out=om, in0=cur, scalar1=-1.0,
                                        scalar2=1.0, op0=OP.mult, op1=OP.add)
                cond = sm.tile([P, 1], f32, tag=f"cond_{tag}",
                               name=f"cond_{tag}")
                nc.vector.tensor_mul(cond, neq, om)
                nc.vector.tensor_mul(cond, cond, yf)
                p1m1 = sm.tile([P, 1], f32, tag=f"p1m1_{tag}",
                               name=f"p1m1_{tag}")
                nc.vector.tensor_scalar_add(p1m1, prob1, -1.0)
                wv = sm.tile([P, 1], f32, tag=f"wv_{tag}", name=f"wv_{tag}")
                nc.vector.tensor_mul(wv, cond, p1m1)
                nc.vector.tensor_scalar_add(wv, wv, 1.0)
                return wv, sameflag

            # ---- weighted masks per group (built during the stream) ----
            s_da = smt[:, 27:28]   # 1 for p<64 (blocks a,b of the diag)
            s_dc = smt[:, 28:29]   # 1 for p<32 or 64<=p<96
            s_off = smt[:, 29:30]  # -2 for p<32
            Ws = []
            for g in range(groups):
                gyf = smt[:, 17 + 9 * g:18 + 9 * g]
                wcg, sameg = weight_chain(smt[:, 9 + 9 * g:11 + 9 * g],
                                          smt[:, 11 + 9 * g:13 + 9 * g],
                                          gyf, f"cf{g}")
                cer = sm.tile([P, 1], f32, tag=f"cer{g}", name=f"cer{g}")
                nc.vector.scalar_tensor_tensor(out=cer, in0=wcg,
                                               scalar=1.0 / (B * HW),
                                               in1=gyf,
                                               op0=OP.mult, op1=OP.mult)
                csp = sm.tile([P, 1], f32, tag=f"csp{g}", name=f"csp{g}")
                nc.vector.scalar_tensor_tensor(out=csp, in0=sameg,
                                               scalar=1.0 / (B * HW),
                                               in1=gyf,
                                               op0=OP.mult, op1=OP.mult)
                cdiag = sm.tile([P, 1], f32, tag=f"cd{g}", name=f"cd{g}")
                nc.vector.tensor_mul(cdiag, cer, s_da)
                t2 = sm.tile([P, 1], f32, tag=f"t2{g}", name=f"t2{g}")
                nc.vector.tensor_mul(t2, csp, s_dc)
                nc.vector.tensor_add(cdiag, cdiag, t2)
                cab = sm.tile([P, 1], f32, tag=f"cab{g}", name=f"cab{g}")
                nc.vector.tensor_mul(cab, cer, s_off)
                cac = sm.tile([P, 1], f32, tag=f"cac{g}", name=f"cac{g}")
                nc.vector.tensor_mul(cac, csp, s_off)

                Wg = sm.tile([GW, GW], f32, tag=f"W{g}", name=f"W{g}")
                scr2 = sm.tile([GW, GW], f32, tag="scr2", name="scr2")
                scr3 = sm.tile([GW, GW], f32, tag="scr3", name="scr3")
                m1 = mask_t[0:GW, 0 * GW:1 * GW]
                m2 = mask_t[0:GW, 1 * GW:2 * GW]
                m3 = mask_t[0:GW, 2 * GW:3 * GW]
                nc.scalar.activation(out=Wg, in_=m1, func=AF.Copy,
                                     scale=cdiag[0:GW])
                nc.scalar.activation(out=scr2, in_=m2, func=AF.Copy,
                                     scale=cab[0:GW])
                nc.scalar.activation(out=scr3, in_=m3, func=AF.Copy,
                                     scale=cac[0:GW])
                nc.vector.tensor_add(Wg, Wg, scr2)
                nc.vector.tensor_add(Wg, Wg, scr3)
                Ws.append(Wg)

            # ---- CE path: per-sample w*(ce+ce_back)/(2B), x2 replicated ----
            def lse2(x0, x1, dd, tag):
                mx = sm.tile([P, 1], f32, tag=f"mx_{tag}", name=f"mx_{tag}")
                nc.vector.tensor_tensor(out=mx, in0=x0, in1=x1, op=OP.max)
                nad = sm.tile([P, 1], f32, tag=f"nad_{tag}",
                              name=f"nad_{tag}")
                nc.vector.tensor_scalar_mul(nad, dd, -1.0)
                nc.vector.tensor_tensor(out=nad, in0=dd, in1=nad, op=OP.min)
                spt = sm.tile([P, 1], f32, tag=f"sp_{tag}", name=f"sp_{tag}")
                nc.scalar.activation(out=spt, in_=nad, func=AF.Exp)
                nc.scalar.activation(out=spt, in_=spt, func=AF.Ln, bias=1.0)
                ls = sm.tile([P, 1], f32, tag=f"ls_{tag}", name=f"ls_{tag}")
                nc.vector.tensor_add(ls, mx, spt)
                return ls

            p1 = smt[:, 0:2]
            p1o = smt[:, 2:4]
            p2 = smt[:, 4:6]
            pb = smt[:, 6:8]
            yf = smt[:, 8:9]
            wv, _ = weight_chain(p1, p1o, yf, "ce")
            d1c = sm.tile([P, 1], f32)
            nc.vector.tensor_sub(d1c, p1[:, 1:2], p1[:, 0:1])
            ls1 = lse2(p1[:, 0:1], p1[:, 1:2], d1c, "p1")
            d2c = sm.tile([P, 1], f32)
            nc.vector.tensor_sub(d2c, p2[:, 1:2], p2[:, 0:1])
            ls2 = lse2(p2[:, 0:1], p2[:, 1:2], d2c, "p2")
            dbc = sm.tile([P, 1], f32)
            nc.vector.tensor_sub(dbc, pb[:, 1:2], pb[:, 0:1])
            lsb = lse2(pb[:, 0:1], pb[:, 1:2], dbc, "pb")

            sel1 = sm.tile([P, 1], f32)
            nc.vector.tensor_mul(sel1, yf, d1c)
            nc.vector.tensor_add(sel1, p1[:, 0:1], sel1)
            ce1 = sm.tile([P, 1], f32)
            nc.vector.tensor_sub(ce1, ls1, sel1)
            sel2 = sm.tile([P, 1], f32)
            nc.vector.tensor_mul(sel2, yf, d2c)
            nc.vector.tensor_add(sel2, p2[:, 0:1], sel2)
            ce2 = sm.tile([P, 1], f32)
            nc.vector.tensor_sub(ce2, ls2, sel2)
            q = sm.tile([P, 1], f32)          # 2*(ce + ce_back)
            nc.vector.tensor_add(q, ce1, ce2)
            cebr = sm.tile([P, 1], f32)
            nc.vector.tensor_sub(cebr, lsb, pb[:, 0:1])
            nc.vector.tensor_mul(cebr, cebr, yf)
            nc.vector.tensor_add(q, q, cebr)
            nc.vector.scalar_tensor_tensor(out=out2[:, 0:1], in0=q,
                                           scalar=1.0 / (4 * B), in1=wv,
                                           op0=OP.mult, op1=OP.mult)

            # ---- tail: G x W row-reduction straight into the output
            # tile; the host finishes the scalar sum ----
            scr = sm.tile([GW, GW], f32)
            for g in range(groups):
                nc.vector.affine_mul_reduce(out=scr,
                                            accum_out=out2[0:GW,
                                                           1 + g:2 + g],
                                            in0=Gs[g], in1=Ws[g],
                                            scale=1.0, bias=0.0)
            nc.sync.dma_start(out=outp, in_=out2)

    nc.compile()
    return nc


def _get_nc(groups):
    if groups not in _NC_CACHE:
        _NC_CACHE[groups] = _build_nc(groups)
    return _NC_CACHE[groups]


def _make_slab(cams1, cams2, idx, sel, groups):
    """[128, MCOLS + groups*96*98] fp8 slab: 288 mask columns, then the
    transposed Gram layout (per 128-HW chunk, 96 columns a|b|c)."""
    out = np.empty((P, MCOLS + groups * PC * NCH),
                   dtype=ml_dtypes.float8_e4m3)
    out[:, 0:MCOLS] = _MASKS
    for g in range(groups):
        sel_g = sel[g * SLOTS:(g + 1) * SLOTS]
        nk = len(sel_g)
        M = np.zeros((GW, HW), dtype=np.float32)
        M[0:nk] = cams1[idx, sel_g, 1].reshape(nk, HW)
        M[SLOTS:SLOTS + nk] = cams2[idx, sel_g, 1].reshape(nk, HW)
        M[2 * SLOTS:2 * SLOTS + nk] = cams1[1 - idx, sel_g, 1].reshape(nk, HW)
        Mq = M.astype(ml_dtypes.float8_e4m3)
        # [96, HW] -> [96, 98, 128] -> [128part, 98, 96col]
        sl = Mq.reshape(GW, NCH, P).transpose(2, 1, 0).reshape(P, PC * NCH)
        out[:, MCOLS + g * PC * NCH:MCOLS + (g + 1) * PC * NCH] = sl
    return out


def _make_static():
    """Per-partition selector columns (f32) + 0/1 mask block (fp8)."""
    st = np.zeros((P, 3), dtype=np.float32)
    p = np.arange(P)
    st[:, 0] = (p < 64).astype(np.float32)
    st[:, 1] = ((p < 32) | ((p >= 64) & (p < 96))).astype(np.float32)
    st[:, 2] = np.where(p < 32, -2.0, 0.0)
    mk = np.zeros((P, MCOLS), dtype=np.float32)
    mk[0:GW, 0:GW] = np.eye(GW, dtype=np.float32)
    r = np.arange(SLOTS)
    mk[r, GW + SLOTS + r] = 1.0
    mk[r, 2 * GW + 2 * SLOTS + r] = 1.0
    return st, mk.astype(ml_dtypes.float8_e4m3)


_STATIC_COLS, _MASKS = _make_static()


def kernel(preds1, cams1, preds1_back, preds2, cams2, y, index):
    from concourse.bass_utils import run_bass_kernel_spmd

    idx = int(np.asarray(index))
    preds1 = np.asarray(preds1, dtype=np.float32)
    preds1_back = np.asarray(preds1_back, dtype=np.float32)
    preds2 = np.asarray(preds2, dtype=np.float32)
    cams1 = np.asarray(cams1, dtype=np.float32)
    cams2 = np.asarray(cams2, dtype=np.float32)
    yi = np.asarray(y).astype(np.int64).reshape(B)
    yf = yi.astype(np.float32).reshape(B, 1)

    sel_all = np.flatnonzero(yi == 1)
    core_sels = [sel_all[(sel_all >= k * BPC) & (sel_all < (k + 1) * BPC)]
                 for k in range(NCORES)]
    # masked path needs <=32 y=1 batches on every core (slots are per-core)
    masked = all(len(sel) <= SLOTS for sel in core_sels)
    if not masked:
        core_sels = [np.arange(k * BPC, (k + 1) * BPC) for k in range(NCORES)]
    groups = 1 if masked else 2
    nc = _get_nc(groups)

    in_maps = []
    for k in range(NCORES):
        s = slice(k * BPC, (k + 1) * BPC)
        sel = core_sels[k]

        sm_host = np.zeros((P, SCOLS), dtype=np.float32)
        ce = np.concatenate(
            [preds1[idx, s], preds1[1 - idx, s], preds2[idx, s],
             preds1_back[idx, s], yf[s]], axis=1)             # [64, 9]
        sm_host[:, 0:9] = np.repeat(ce, 2, axis=0)
        for g in range(groups):
            sel_g = sel[g * SLOTS:(g + 1) * SLOTS]
            nk = len(sel_g)
            cf = np.zeros((SLOTS, 9), dtype=np.float32)
            cf[0:nk] = np.concatenate(
                [preds1[idx, sel_g], preds1[1 - idx, sel_g],
                 preds2[idx, sel_g], preds1_back[idx, sel_g],
                 yf[sel_g]], axis=1)
            sm_host[0:GW, 9 + 9 * g:18 + 9 * g] = np.tile(cf, (3, 1))
        sm_host[:, 27:30] = _STATIC_COLS

        im = {
            "small": sm_host,
            "slab": _make_slab(cams1, cams2, idx, sel, groups),
        }
        in_maps.append(im)

    trace = bool(int(os.environ.get("KERNEL_TRACE", "0")))
    res = run_bass_kernel_spmd(nc, in_maps, core_ids=list(range(NCORES)),
                               trace=trace)
    kernel.last_exec_time_ns = res.exec_time_ns
    total = sum(float(res.results[k]["out"].sum()) for k in range(NCORES))
    return np.array(total, dtype=np.float32)


kernel.last_exec_time_ns = None
